# revision 2
# baseline (speedup 1.0000x reference)
"""Causal multi-head attention (B=4, H=16, S=2048, D=64) on 8 Trainium2 cores.

Sharding: B*H = 64 independent attention problems, 8 heads per core.

Per-core kernel design (all matmuls fp32r, 1 cycle/row at N>=256):
- Heads processed in pairs (A at partitions 0:64, B at 64:128).
- Q,K loaded tile-interleaved [128, (tile, head, d)] so one [128,128] PE
  transpose yields both heads' Q^T/K^T stacked -> qt/kt [128, 2048].
- S^T = K @ Q^T computed directly in [k, q] layout: lhsT = K^T chunk [64,128],
  rhs = Q^T block [64, 512]; the two heads' matmuls run concurrently on
  disjoint PE row groups (base partition 0 / 64).
- exp via one ACT instruction per 4-bank PSUM group [128, 2048] (scale=1/8
  folded into the activation); causal masking via multiplicative 0/1 masks
  on diagonal chunks only.
- PV: lhsT = V_aug [128k, 65] (ones column appended -> row sums for free),
  rhs = P^T [128k, 512q], accumulated over k chunks in PSUM [65, 512].
- Finalize: PE-transpose [65,512] -> 4x [128,65], reciprocal of the l column,
  per-partition scalar multiply, contiguous DMA out.
"""
import numpy as np

B, H, S, D = 4, 16, 2048, 64
NCORES = 8
HPC = B * H // NCORES      # 8 heads per core
P = 128
QBLK = 512
NT = S // P                # 16 k-chunks / q-tiles per head
NBLK = S // QBLK           # 4 q blocks
NPAIR = HPC // 2           # 4 head pairs per core

_cache = {}


def _build():
    from contextlib import ExitStack
    import concourse.bacc as bacc
    import concourse.tile as tile
    import concourse.mybir as mybir
    from concourse.masks import make_identity

    f32 = mybir.dt.float32
    f32r = mybir.dt.float32r
    AF = mybir.ActivationFunctionType

    nc = bacc.Bacc("TRN2", target_bir_lowering=False, debug=False,
                   num_devices=NCORES)
    Qd = nc.dram_tensor("Q", (HPC, S, D), f32, kind="ExternalInput")
    Kd = nc.dram_tensor("K", (HPC, S, D), f32, kind="ExternalInput")
    Vd = nc.dram_tensor("V", (HPC, S, D), f32, kind="ExternalInput")
    Od = nc.dram_tensor("O", (HPC, S, D), f32, kind="ExternalOutput")

    with tile.TileContext(nc) as tc, ExitStack() as ctx:
        consts = ctx.enter_context(tc.tile_pool(name="consts", bufs=1))
        raw = ctx.enter_context(tc.tile_pool(name="raw", bufs=2))
        qk = ctx.enter_context(tc.tile_pool(name="qk", bufs=2))
        ptp = ctx.enter_context(tc.tile_pool(name="ptp", bufs=2))
        fin = ctx.enter_context(tc.tile_pool(name="fin", bufs=2))
        st_ps = ctx.enter_context(tc.tile_pool(name="st_ps", bufs=1, space="PSUM"))
        acc_ps = ctx.enter_context(tc.tile_pool(name="acc_ps", bufs=2, space="PSUM"))
        tr_ps = ctx.enter_context(tc.tile_pool(name="tr_ps", bufs=2, space="PSUM"))

        ident = consts.tile([128, 128], f32)
        make_identity(nc, ident[:])
        onesf = consts.tile([128, 2 * NT], f32)
        nc.vector.memset(onesf[:], 1.0)
        # masks[j][r, c] = 1.0 iff c - r - j*128 >= 0 (valid); only cols
        # 0:(j+1)*128 are ever used (cols beyond are fully valid).
        masks = []
        for j in range(4):
            mj = consts.tile([128, 512], f32, name=f"mask{j}")
            nc.gpsimd.memset(mj[:], 1.0)
            nc.gpsimd.affine_select(
                out=mj[:], in_=mj[:], compare_op=mybir.AluOpType.is_ge,
                fill=0.0, base=-128 * j, pattern=[[1, 512]],
                channel_multiplier=-1)
            masks.append(mj)

        for pair in range(NPAIR):
            hA = 2 * pair
            # ---- loads ----
            qraw = raw.tile([128, NT * 2 * 64], f32)
            kraw = raw.tile([128, NT * 2 * 64], f32)
            vf = raw.tile([128, 2 * NT * 64], f32)
            for hh in range(2):
                nc.sync.dma_start(
                    qraw[:].rearrange("p (n h d) -> p n h d", n=NT, h=2)[:, :, hh, :],
                    Qd[hA + hh, :, :].rearrange("(n p) d -> p n d", p=P))
                nc.sync.dma_start(
                    kraw[:].rearrange("p (n h d) -> p n h d", n=NT, h=2)[:, :, hh, :],
                    Kd[hA + hh, :, :].rearrange("(n p) d -> p n d", p=P))
                nc.sync.dma_start(
                    vf[:].rearrange("p (h n d) -> p h n d", h=2, n=NT)[:, hh, :, :],
                    Vd[hA + hh, :, :].rearrange("(n p) d -> p n d", p=P))
            vsb = raw.tile([128, 2 * NT * 65], f32r)
            nc.vector.tensor_copy(
                vsb[:].rearrange("p (h n e) -> p h n e", h=2, n=NT)[:, :, :, 0:64],
                vf[:].rearrange("p (h n d) -> p h n d", h=2, n=NT))
            nc.vector.tensor_copy(
                vsb[:].rearrange("p (h n e) -> p h n e", h=2, n=NT)[:, :, :, 64:65],
                onesf[:].rearrange("p (h n) -> p h n", h=2)[:, :, :, None])
            vv = vsb[:].rearrange("p (h n e) -> p h n e", h=2, n=NT)

            # ---- stacked transposes: qt/kt rows 0:64 = head A, 64:128 = B ----
            qt = qk.tile([128, S], f32r)
            kt = qk.tile([128, S], f32r)
            for t in range(NT):
                for src, dst in ((qraw, qt), (kraw, kt)):
                    tp = tr_ps.tile([128, 128], f32, tag="tr", name="tp")
                    nc.tensor.transpose(
                        tp[:],
                        src[:].rearrange("p (n c) -> p n c", n=NT)[:, t, :],
                        ident[:])
                    nc.vector.tensor_copy(dst[:, t * 128:(t + 1) * 128], tp[:])

            # ---- attention blocks ----
            for b in range(NBLK):
                nchunks = 4 * b + 4
                accs = [acc_ps.tile([65, 512], f32, tag="acc", name=f"acc{hh}")
                        for hh in range(2)]
                for g in range(nchunks // 2):
                    cpair = (2 * g, 2 * g + 1)
                    st = st_ps.tile([128, 2048], f32, tag="st", name="st")
                    quads = [(cc, hh) for cc in cpair for hh in range(2)]
                    for i, (cc, hh) in enumerate(quads):
                        nc.tensor.matmul(
                            st[:, i * 512:(i + 1) * 512],
                            kt[hh * 64:(hh + 1) * 64, cc * 128:(cc + 1) * 128],
                            qt[hh * 64:(hh + 1) * 64, b * 512:(b + 1) * 512],
                            start=True, stop=True)
                    pt = ptp.tile([128, 2048], f32r, tag="pt", name="pt")
                    nc.scalar.activation(pt[:], st[:], AF.Exp, scale=0.125)
                    for i, (cc, hh) in enumerate(quads):
                        j = cc - 4 * b
                        if j >= 0:  # diagonal chunk: zero invalid region
                            w = (j + 1) * 128
                            nc.vector.tensor_mul(
                                pt[:, i * 512:i * 512 + w],
                                pt[:, i * 512:i * 512 + w],
                                masks[j][:, 0:w])
                    for i, (cc, hh) in enumerate(quads):
                        nc.tensor.matmul(
                            accs[hh][:],
                            vv[:, hh, cc, :],
                            pt[:, i * 512:(i + 1) * 512],
                            start=(cc == 0), stop=(cc == nchunks - 1))

                # ---- finalize block: transpose + normalize + store ----
                for hh in range(2):
                    osb = fin.tile([65, 512], f32, tag="osb", name="osb")
                    nc.vector.tensor_copy(osb[:], accs[hh][:])
                    ot = tr_ps.tile([128, 260], f32, tag="tr", name="ot")
                    for j in range(4):
                        nc.tensor.transpose(
                            ot[:, j * 65:(j + 1) * 65],
                            osb[:, j * 128:(j + 1) * 128],
                            ident[0:65, 0:65])
                    recip = fin.tile([128, 4], f32, tag="recip", name="recip")
                    nc.vector.reciprocal(
                        recip[:],
                        ot[:].rearrange("p (j e) -> p j e", e=65)[:, :, 64])
                    o_sb = fin.tile([128, 256], f32, tag="o_sb", name="o_sb")
                    for j in range(4):
                        nc.vector.tensor_scalar_mul(
                            o_sb[:, j * 64:(j + 1) * 64],
                            ot[:, j * 65:j * 65 + 64],
                            recip[:, j:j + 1])
                    nc.sync.dma_start(
                        Od[hA + hh, b * 512:(b + 1) * 512, :]
                        .rearrange("(s p) d -> p s d", p=P),
                        o_sb[:].rearrange("p (s d) -> p s d", s=4))

    nc.compile()
    return nc


def _get_nc():
    if "nc" not in _cache:
        _cache["nc"] = _build()
    return _cache["nc"]


def kernel(Q, K, V):
    from concourse.bass_utils import run_bass_kernel_spmd

    Q = np.ascontiguousarray(np.asarray(Q, dtype=np.float32)).reshape(B * H, S, D)
    K = np.ascontiguousarray(np.asarray(K, dtype=np.float32)).reshape(B * H, S, D)
    V = np.ascontiguousarray(np.asarray(V, dtype=np.float32)).reshape(B * H, S, D)

    nc = _get_nc()
    in_maps = [
        {"Q": Q[c * HPC:(c + 1) * HPC],
         "K": K[c * HPC:(c + 1) * HPC],
         "V": V[c * HPC:(c + 1) * HPC]}
        for c in range(NCORES)
    ]
    res = run_bass_kernel_spmd(nc, in_maps, core_ids=list(range(NCORES)))
    out = np.concatenate([res.results[c]["O"] for c in range(NCORES)], axis=0)
    return out.reshape(B, H, S, D)
